# revision 1
# baseline (speedup 1.0000x reference)
"""Multi-head attention layer (N=4, L=S=2048, D=1024, H=16) on 8 TRN2 NeuronCores.

Sharding: 8 cores = 4 batches x 2 query-halves (heads kept local, so no
collectives: each core computes Q projection for its 1024 query rows, K/V
projections for the full 2048 keys of its batch, all 16 heads of attention,
and the output projection for its rows). Host shards/gathers.

Per-core data layout (host-prepared, bf16):
  xq [128, 8, 1024]  xq[p,t,l] = queries[n, l0+l, t*128+p]   (transposed)
  xk/xv [128, 8, 2048]  keys[n].T / values[n].T, same packing
  wq/wk/wv/wo [128, 8, 1024]  w[p,t,d] = W[t*128+p, d]
  bq/bk [128, 8] f32; bv [64, 16] f32; bo [128, 1024] f32 (pre-broadcast)
  out [1024, 1024] f32 (natural layout)

Schedule notes (v2 — PE-dense from ~8us):
- The PE engine is the bottleneck (~397us of matmul slots). The schedule
  keeps it dense from the first K-projection: attention pair (dt=0,lb=0)
  starts as soon as K(0,0)+Q(0,0) are projected (~8us); ALL other
  projections (V, later K/Q d-tiles, O) ride as PE filler inside the
  attention units via a pending-queue feed (one group per st slot, ~2
  groups of DMA prefetch).
- PV matmuls LAG the scores/exp stream (exp tiles buffer in SBUF): PV of
  (unit, st) is emitted only once its V-projection group exists and the
  exp is emitted. Only the oldest unit holds open PV accumulators (the
  pepo pool has exactly one unit's pair of banks).
- The softmax denominator comes free from a ones-column at slot 0 of the
  augmented V (PV out row 0 = sum of exp); normalization broadcasts the
  denominator from partition 0 via gpsimd and multiplies off the critical
  path. attn rows sum to 1, so V's bias is added after normalization.
- DMA issue order is by need-time: K(0)/Q(0) inputs, then xv0+wv-half for
  the first V groups, then xk chunks, then remaining weights.
- Final O-projection groups are pairwise-interleaved so their ct0-6
  matmuls run while the last unit's normalize is in flight.
"""

import numpy as np
import ml_dtypes

import concourse.bass as bass
import concourse.mybir as mybir
import concourse.tile as tile
from concourse import bacc
from concourse.bass_utils import run_bass_kernel_spmd

BF16 = mybir.dt.bfloat16
F32 = mybir.dt.float32
ALU = mybir.AluOpType
ACTF = mybir.ActivationFunctionType

N, L, S, D, H, E = 4, 2048, 2048, 1024, 16, 64
LQ = 1024
N_CORES = 8

_nc_cache = None
last_results = None


def _build():
    nc = bacc.Bacc(None, target_bir_lowering=False)

    xq = nc.declare_dram_parameter("xq", [128, 8, LQ], BF16, isOutput=False)
    xk = nc.declare_dram_parameter("xk", [128, 8, S], BF16, isOutput=False)
    xv = nc.declare_dram_parameter("xv", [128, 8, S], BF16, isOutput=False)
    wq = nc.declare_dram_parameter("wq", [128, 8, D], BF16, isOutput=False)
    wk = nc.declare_dram_parameter("wk", [128, 8, D], BF16, isOutput=False)
    wv = nc.declare_dram_parameter("wv", [128, 8, D], BF16, isOutput=False)
    wo = nc.declare_dram_parameter("wo", [128, 8, D], BF16, isOutput=False)
    bq = nc.declare_dram_parameter("bq", [128, 8], F32, isOutput=False)
    bk = nc.declare_dram_parameter("bk", [128, 8], F32, isOutput=False)
    bv = nc.declare_dram_parameter("bv", [64, 16], F32, isOutput=False)
    bo = nc.declare_dram_parameter("bo", [128, D], F32, isOutput=False)
    out = nc.declare_dram_parameter("out", [LQ, D], F32, isOutput=True)

    with tile.TileContext(nc) as tc:
        with tc.tile_pool(name="const", bufs=1) as cpool, \
             tc.tile_pool(name="pers", bufs=1) as ppool, \
             tc.tile_pool(name="stage", bufs=3) as spool, \
             tc.tile_pool(name="work", bufs=2) as wpool, \
             tc.tile_pool(name="expp", bufs=5) as epool, \
             tc.tile_pool(name="psum", bufs=2, space="PSUM") as psum:

            wq_t = cpool.tile([128, 8, D], BF16, tag="w_a")
            wk_t = cpool.tile([128, 8, D], BF16, tag="w_b")
            wv_t = cpool.tile([128, 8, D], BF16, tag="w_c")
            bq_t = cpool.tile([128, 8], F32, tag="bq")
            bk_t = cpool.tile([128, 8], F32, tag="bk")
            bv_t = cpool.tile([64, 16], F32, tag="bv")
            bo_t = cpool.tile([128, D], F32, tag="bo")
            qT = ppool.tile([128, 8, LQ], BF16, tag="qT")
            kT = ppool.tile([128, 8, S], BF16, tag="kT")
            vaug = ppool.tile([128, 16, 16 * 65], BF16, tag="vaug")
            oT = ppool.tile([128, 8, LQ], BF16, tag="oT")

            # ---- critical-path DMAs: K(0,0) + Q(0,0) inputs first ----
            nc.sync.dma_start(wk_t[:, :, 0:128], wk[:, :, 0:128])
            sgk0 = spool.tile([128, 8, 512], BF16, tag="stage")
            nc.sync.dma_start(sgk0[:], xk[:, :, 0:512])
            nc.sync.dma_start(wq_t[:, :, 0:128], wq[:, :, 0:128])
            sgq0 = spool.tile([128, 8, 512], BF16, tag="stage")
            nc.sync.dma_start(sgq0[:], xq[:, :, 0:512])
            nc.sync.dma_start(bk_t[:], bk[:])
            nc.sync.dma_start(bq_t[:], bq[:])

            # warm the exp table-set while DMAs fill (one tiny ACTIVATE)
            wrm = wpool.tile([1, 16], F32, tag="warm")
            nc.vector.memset(wrm[:], 0.0)
            wrm2 = wpool.tile([1, 16], F32, tag="warm2")
            nc.scalar.activation(wrm2[:], wrm[:], ACTF.Exp, scale=0.125)

            # ones column (slot 64) of augmented V => PV row 64 = softmax denom
            for st in range(16):
                v3 = vaug[:, st].rearrange("p (h e) -> p h e", e=65)
                nc.vector.memset(v3[:, :, 64:65], 1.0)

            # ---- projection-group emitters ----
            def proj_group(w_t, sg_t, dt, dst, bias):
                ps = psum.tile([128, 512], F32, tag="mm512", bufs=2)
                for ct in range(8):
                    nc.tensor.matmul(ps[:], w_t[:, ct, dt * 128:(dt + 1) * 128],
                                     sg_t[:, ct, :], start=(ct == 0),
                                     stop=(ct == 7))
                nc.vector.tensor_scalar_add(dst, ps[:], bias)

            def v_proj_group(sg_t, stl, st, db):
                ps = psum.tile([128, 512], F32, tag="mm512", bufs=2)
                for ct in range(8):
                    nc.tensor.matmul(ps[:], sg_t[:, ct, stl * 128:(stl + 1) * 128],
                                     wv_t[:, ct, db * 512:(db + 1) * 512],
                                     start=(ct == 0), stop=(ct == 7))
                v3 = vaug[:, st].rearrange("p (h e) -> p h e", e=65)
                nc.vector.tensor_copy(
                    v3[:, db * 8:(db + 1) * 8, 0:64],
                    ps[:].rearrange("p (h e) -> p h e", e=64))

            def o_proj_group(lt, db):
                ps = psum.tile([128, 512], F32, tag="mm512", bufs=2)
                for ct in range(8):
                    nc.tensor.matmul(ps[:], oT[:, ct, lt * 128:(lt + 1) * 128],
                                     wo_t[:, ct, db * 512:(db + 1) * 512],
                                     start=(ct == 0), stop=(ct == 7))
                ob = wpool.tile([128, 512], F32, tag="outsb")
                nc.vector.tensor_add(ob[:], ps[:],
                                     bo_t[:, db * 512:(db + 1) * 512])
                nc.sync.dma_start(
                    out[lt * 128:(lt + 1) * 128, db * 512:(db + 1) * 512], ob[:])

            # ---- feed machinery ----
            emitted = set()        # keys of emitted groups
            v_ready = [0, 0]       # per db: count of emitted V st-groups
            norm_done = [0, 0]     # per lb: count of fully-normalized units

            k_box, q_box, v_box = [None], [None], [None]

            def k_item(dt, sb, fresh=False):
                def dma():
                    if fresh:
                        sg = spool.tile([128, 8, 512], BF16, tag="stage")
                        nc.sync.dma_start(sg[:],
                                          xk[:, :, sb * 512:(sb + 1) * 512])
                        k_box[0] = sg
                    return k_box[0]
                def compute(sg):
                    proj_group(wk_t, sg, dt, kT[:, dt, sb * 512:(sb + 1) * 512],
                               bk_t[:, dt:dt + 1])
                    emitted.add(("k", dt, sb))
                return (dma, compute, ("k", dt, sb))

            def q_item(dt, lb, fresh=False):
                def dma():
                    if fresh:
                        sg = spool.tile([128, 8, 512], BF16, tag="stage")
                        nc.sync.dma_start(sg[:],
                                          xq[:, :, lb * 512:(lb + 1) * 512])
                        q_box[0] = sg
                    return q_box[0]
                def compute(sg):
                    proj_group(wq_t, sg, dt, qT[:, dt, lb * 512:(lb + 1) * 512],
                               bq_t[:, dt:dt + 1])
                    emitted.add(("q", dt, lb))
                return (dma, compute, ("q", dt, lb))

            def v_item(db, st, fresh=False):
                sb, stl = st // 4, st % 4
                def dma():
                    if fresh:
                        sg = spool.tile([128, 8, 512], BF16, tag="stage")
                        nc.sync.dma_start(sg[:],
                                          xv[:, :, sb * 512:(sb + 1) * 512])
                        v_box[0] = sg
                    return v_box[0]
                def compute(sg):
                    v_proj_group(sg, stl, st, db)
                    v_ready[db] += 1
                    emitted.add(("v", db, st))
                return (dma, compute, ("v", db, st))

            def dma_feed(fn, name):
                return (None, lambda sg: fn(), ("w", name, 0))

            def o_item(lt, db):
                def compute(sg):
                    o_proj_group(lt, db)
                    emitted.add(("o", lt, db))
                return (None, compute, ("o", lt, db))

            def free_items():
                for st in range(1, 4):          # preload did v(0,0) fresh
                    yield v_item(0, st)
                yield k_item(1, 0, fresh=True)
                yield k_item(2, 0)
                for st in range(4, 8):
                    yield v_item(0, st, fresh=(st == 4))
                yield q_item(1, 0, fresh=True)
                for d in range(2, 4):
                    yield q_item(d, 0)
                yield k_item(1, 1, fresh=True)
                yield k_item(2, 1)
                for st in range(8, 12):
                    yield v_item(0, st, fresh=(st == 8))
                yield k_item(1, 2, fresh=True)
                yield k_item(2, 2)
                yield k_item(1, 3, fresh=True)
                yield k_item(2, 3)
                for st in range(12, 16):
                    yield v_item(0, st, fresh=(st == 12))
                yield k_item(3, 0, fresh=True)
                yield k_item(4, 0)
                yield k_item(3, 1, fresh=True)
                yield k_item(4, 1)
                yield dma_feed(lambda: nc.sync.dma_start(
                    wv_t[:, :, 512:1024], wv[:, :, 512:1024]), "wv1")
                yield k_item(3, 2, fresh=True)
                yield k_item(4, 2)
                yield k_item(3, 3, fresh=True)
                yield k_item(4, 3)
                for st in range(0, 4):
                    yield v_item(1, st, fresh=(st == 0))
                for st in range(4, 8):
                    yield v_item(1, st, fresh=(st == 4))
                yield dma_feed(lambda: nc.sync.dma_start(
                    wk_t[:, :, 640:1024], wk[:, :, 640:1024]), "wk57")
                yield dma_feed(lambda: nc.sync.dma_start(
                    wq_t[:, :, 512:1024], wq[:, :, 512:1024]), "wq47")
                for st in range(8, 12):
                    yield v_item(1, st, fresh=(st == 8))
                yield q_item(4, 0, fresh=True)
                for d in range(5, 8):
                    yield q_item(d, 0)
                for st in range(12, 16):
                    yield v_item(1, st, fresh=(st == 12))
                yield dma_feed(lambda: nc.sync.dma_start(bo_t[:], bo[:]),
                               "bo")
                for sb in range(4):
                    for d in range(5, 8):
                        yield k_item(d, sb, fresh=(d == 5))
                for d in range(8):
                    yield q_item(d, 1, fresh=(d == 0))
                for lt in range(4):
                    for db in range(2):
                        yield o_item(lt, db)

            feed = free_items()
            feed_buf = []          # peeked-but-gated items
            pending = []           # (compute, sg, key): DMA issued, not emitted

            def item_ok(it):
                key = it[2]
                return key[0] != "o" or norm_done[0] >= 8

            def emit_one():
                compute, sg, key = pending.pop(0)
                compute(sg)
                return key

            def pull():
                if feed_buf:
                    it = feed_buf[0]
                    if not item_ok(it):
                        return False
                    feed_buf.pop(0)
                else:
                    it = next(feed, None)
                    if it is None:
                        return False
                    if not item_ok(it):
                        feed_buf.append(it)
                        return False
                dma, compute, key = it
                sg = dma() if dma else None
                pending.append((compute, sg, key))
                return True

            def pump():
                while len(pending) < 3:
                    if not pull():
                        break
                if len(pending) > 2:
                    emit_one()

            def ensure(kind, a, b):
                key = (kind, a, b)
                while key not in emitted:
                    if pending:
                        emit_one()
                    elif not pull():
                        raise RuntimeError(f"cannot satisfy prereq {key}")

            # ---- attention units with lagging PV ----
            units = []

            def drain_pv():
                while units:
                    u = units[0]
                    while (u["next"] < 16 and u["next"] < v_ready[u["db"]]
                           and u["next"] < u["exp_n"]):
                        st = u["next"]
                        if st == 0:
                            u["pe"] = psum.tile([128, 512], F32, tag="pepo",
                                                bufs=2, name="pe_acc")
                            u["po"] = psum.tile([128, 512], F32, tag="pepo",
                                                bufs=2, name="po_acc")
                        he, ho = 2 * u["dt"], 2 * u["dt"] + 1
                        ep = u["ep"][st]
                        nc.tensor.matmul(u["pe"][0:65, :],
                                         vaug[:, st, he * 65:(he + 1) * 65],
                                         ep[:, 0:512],
                                         start=(st == 0), stop=(st == 15))
                        nc.tensor.matmul(u["po"][0:65, :],
                                         vaug[:, st, ho * 65:(ho + 1) * 65],
                                         ep[:, 512:1024],
                                         start=(st == 0), stop=(st == 15))
                        u["ep"][st] = None
                        u["next"] += 1
                    if u["next"] == 16:
                        unit_epilogue(u)
                        units.pop(0)
                    else:
                        break

            def normalize(cp, h, lb):
                # cp: [65, 512] f32 SBUF; row 64 = softmax denominator
                den0 = wpool.tile([1, 512], F32, tag="den0")
                nc.sync.dma_start(den0[0:1, :], cp[64:65, :])
                recb = wpool.tile([64, 512], F32, tag="recb")
                nc.gpsimd.partition_broadcast(recb[:], den0[0:1, :])
                nc.vector.reciprocal_approx_fast(recb[:], recb[:])
                dt = h // 2
                if h % 2 == 0:
                    dst = oT[0:64, dt, lb * 512:(lb + 1) * 512]
                    nc.vector.tensor_tensor(dst, cp[0:64, :], recb[:], ALU.mult)
                    nc.vector.tensor_scalar_add(dst, dst, bv_t[:, h:h + 1])
                else:
                    tmp = wpool.tile([64, 512], BF16, tag="otmp")
                    nc.vector.tensor_tensor(tmp[:], cp[0:64, :], recb[:],
                                            ALU.mult)
                    nc.vector.tensor_scalar_add(tmp[:], tmp[:], bv_t[:, h:h + 1])
                    nc.sync.dma_start(
                        oT[64:128, dt, lb * 512:(lb + 1) * 512], tmp[:])

            def unit_epilogue(u):
                dt, lb = u["dt"], u["lb"]
                cpe = wpool.tile([65, 512], F32, tag="cpe")
                nc.vector.tensor_copy(cpe[:], u["pe"][0:65, :])
                cpo = wpool.tile([65, 512], F32, tag="cpo")
                nc.vector.tensor_copy(cpo[:], u["po"][0:65, :])
                normalize(cpe, 2 * dt, lb)
                normalize(cpo, 2 * dt + 1, lb)
                norm_done[lb] += 1

            def attention_unit(dt, lb):
                ensure("q", dt, lb)
                ensure("k", dt, 0)
                u = {"dt": dt, "lb": lb, "db": dt // 4, "next": 0, "exp_n": 0,
                     "ep": [None] * 16}
                units.append(u)
                qe = qT[0:64, dt, lb * 512:(lb + 1) * 512]
                qo = qT[64:128, dt, lb * 512:(lb + 1) * 512]
                for st in range(16):
                    if st % 4 == 0 and st > 0:
                        ensure("k", dt, st // 4)
                    ps2 = psum.tile([128, 1024], F32, tag="sc2", bufs=2)
                    nc.tensor.matmul(ps2[:, 0:512],
                                     kT[0:64, dt, st * 128:(st + 1) * 128],
                                     qe, start=True, stop=True)
                    nc.tensor.matmul(ps2[:, 512:1024],
                                     kT[64:128, dt, st * 128:(st + 1) * 128],
                                     qo, start=True, stop=True)
                    ep = epool.tile([128, 1024], BF16, tag="ep")
                    nc.scalar.activation(ep[:], ps2[:], ACTF.Exp, scale=0.125)
                    u["ep"][st] = ep
                    u["exp_n"] = st + 1
                    pump()
                    drain_pv()

            # ---- startup: K(0,0)/Q(0,0) immediately, then prefetch ----
            kd, kc, _ = k_item(0, 0)
            kc(sgk0)
            qd, qc, _ = q_item(0, 0)
            qc(sgq0)

            # DMA issue order = need order: xk1 for K(0,1), wv half + xv0
            # for the first V groups, then xk2/xk3 and remaining weights
            it = k_item(0, 1, fresh=True)
            pending.append((it[1], it[0](), it[2]))
            nc.sync.dma_start(wv_t[:, :, 0:512], wv[:, :, 0:512])
            it = v_item(0, 0, fresh=True)
            pending.append((it[1], it[0](), it[2]))
            for sb in (2, 3):
                it = k_item(0, sb, fresh=True)
                pending.append((it[1], it[0](), it[2]))
            nc.sync.dma_start(wq_t[:, :, 128:512], wq[:, :, 128:512])
            nc.sync.dma_start(wk_t[:, :, 128:640], wk[:, :, 128:640])
            nc.sync.dma_start(bv_t[:], bv[:])

            # ---- attention sweeps ----
            for lb in range(2):
                for dt in range(8):
                    attention_unit(dt, lb)
                if lb == 0:
                    # wo reuses wq's slot: every q group must be emitted first
                    for d in range(8):
                        ensure("q", d, 1)
                    wo_t = cpool.tile([128, 8, D], BF16, tag="w_a")
                    nc.sync.dma_start(wo_t[:], wo[:])

            # ---- flush: finish lagging PVs, then any remaining filler ----
            for _ in range(4096):
                drain_pv()
                if pending:
                    emit_one()
                elif not pull():
                    if not units and not feed_buf:
                        break
            assert not units and not pending and not feed_buf

            # final O-proj groups: six accumulators (mm512 x2, free sc2
            # halves x2, free pepo x2) run their ct0-6 matmuls during the
            # last unit's normalize; only ct7 + epilogue (and two full
            # groups) remain after it.
            finals = [(lt, db) for lt in range(4, 8) for db in range(2)]
            accs = []
            for i, (lt, db) in enumerate(finals[:6]):
                if i < 2:
                    ps = psum.tile([128, 512], F32, tag="mm512", bufs=2,
                                   name="oaccA")
                    ap = ps[:]
                elif i < 4:
                    ps = psum.tile([128, 1024], F32, tag="sc2", bufs=2,
                                   name="oaccB")
                    ap = ps[:, 0:512]
                else:
                    ps = psum.tile([128, 512], F32, tag="pepo", bufs=2,
                                   name="oaccC")
                    ap = ps[:]
                for ct in range(7):
                    nc.tensor.matmul(
                        ap, oT[:, ct, lt * 128:(lt + 1) * 128],
                        wo_t[:, ct, db * 512:(db + 1) * 512],
                        start=(ct == 0), stop=False)
                accs.append(ap)
            for (lt, db), ap in zip(finals[:6], accs):
                nc.tensor.matmul(
                    ap, oT[:, 7, lt * 128:(lt + 1) * 128],
                    wo_t[:, 7, db * 512:(db + 1) * 512],
                    start=False, stop=True)
                ob = wpool.tile([128, 512], F32, tag="outsb")
                nc.vector.tensor_add(ob[:], ap,
                                     bo_t[:, db * 512:(db + 1) * 512])
                nc.sync.dma_start(
                    out[lt * 128:(lt + 1) * 128,
                        db * 512:(db + 1) * 512], ob[:])
            for lt, db in finals[6:]:
                o_proj_group(lt, db)

    nc.compile()
    return nc


def _pack_kxm(w):
    k, m = w.shape
    return np.ascontiguousarray(
        w.reshape(k // 128, 128, m).transpose(1, 0, 2)).astype(ml_dtypes.bfloat16)


def kernel(queries, keys, values, Wq, bq, Wk, bk, Wv, bv, Wo, bo):
    global _nc_cache, last_results
    queries = np.asarray(queries, dtype=np.float32)
    keys = np.asarray(keys, dtype=np.float32)
    values = np.asarray(values, dtype=np.float32)

    if _nc_cache is None:
        _nc_cache = _build()
    nc = _nc_cache

    w_packed = {
        "wq": _pack_kxm(np.asarray(Wq, np.float32)),
        "wk": _pack_kxm(np.asarray(Wk, np.float32)),
        "wv": _pack_kxm(np.asarray(Wv, np.float32)),
        "wo": _pack_kxm(np.asarray(Wo, np.float32)),
        "bq": np.ascontiguousarray(np.asarray(bq, np.float32).reshape(8, 128).T),
        "bk": np.ascontiguousarray(np.asarray(bk, np.float32).reshape(8, 128).T),
        "bv": np.ascontiguousarray(np.asarray(bv, np.float32).reshape(16, 64).T),
        "bo": np.ascontiguousarray(
            np.broadcast_to(np.asarray(bo, np.float32), (128, D))),
    }

    in_maps = []
    for c in range(N_CORES):
        n, half = c // 2, c % 2
        m = dict(w_packed)
        m["xq"] = _pack_kxm(
            np.ascontiguousarray(queries[n, half * LQ:(half + 1) * LQ, :].T))
        m["xk"] = _pack_kxm(np.ascontiguousarray(keys[n].T))
        m["xv"] = _pack_kxm(np.ascontiguousarray(values[n].T))
        in_maps.append(m)

    last_results = run_bass_kernel_spmd(nc, in_maps, list(range(N_CORES)))

    full = np.empty((N, L, D), np.float32)
    for c in range(N_CORES):
        n, half = c // 2, c % 2
        full[n, half * LQ:(half + 1) * LQ, :] = last_results.results[c]["out"]
    return full

